# revision 9
# baseline (speedup 1.0000x reference)
"""MiniMaxText01 linear attention on 8 trn2 NeuronCores — per-head mixed fp8.

Sharding: core c -> batch b = c//4, quad qd = c%4. Head rebalance: core slots
hold global heads [2qd, 2qd+1, 8+qd, 12+qd] so every core has the same
precision pattern (out_proj partials sum over cores, so head order is free).

Precision (chosen from a numpy error model of the full pipeline, metric
max|err|/max|ref| vs f64, budget 2e-2):
  k,v projections : fp8-e4m3 DoubleRow, all 4 slots       (err ~1.0e-2)
  q,gate          : fp8 slots 0-2, fp16 slot 3 (the slow-decay head whose
                    large kv state amplifies quantization error)
  out_proj        : fp8 DoubleRow for the slot-0/1 pair (heads 0-7, tiny
                    attn magnitude), fp16 for slots 2-3; go carries a x2
                    pre-scale (folded into the decay tables) and wo a x256
                    scale so both pairs accumulate at psum scale 512
  attn            : fp16
fp8 operands are host-quantized with power-of-2 scales (x*16, w*256; e4m3 max
on TRN is 240) divided out by the fused activation that drains PSUM.
"""

import numpy as np

B, S, HID = 2, 4096, 2048
H, D, C = 16, 128, 256
BLK = 512              # token block (2 chunks)
NBLK = S // BLK        # 8 blocks
KO = HID // 128        # 16 contraction subtiles
KK = KO // 2           # 8 DoubleRow pairs
NCORES = 8
P = 128

SX = 16.0              # x fp8 scale
SW = 256.0             # weight fp8 scale
SGO = 2.0              # go pre-scale (folded into ddp/qdt)
DESC = 1.0 / (SX * SW)
DESC_O = 1.0 / (SGO * SW)

_PROG = None


def _build_program():
    import concourse.bacc as bacc
    import concourse.mybir as mybir
    import concourse.tile as tile

    F32 = mybir.dt.float32
    F16 = mybir.dt.float16
    F8 = mybir.dt.float8e4
    AF = mybir.ActivationFunctionType
    MUL = mybir.AluOpType.mult
    ADD = mybir.AluOpType.add
    DR = mybir.MatmulPerfMode.DoubleRow

    nc = bacc.Bacc("TRN2", target_bir_lowering=False, debug=False,
                   num_devices=NCORES)

    # x pre-tiled on host: [blk, kp, ko, t]
    xT8 = nc.dram_tensor("xT8", [NBLK, P, KO, BLK], F8, kind="ExternalInput")
    xT16 = nc.dram_tensor("xT16", [NBLK, P, KO, BLK], F16, kind="ExternalInput")
    # weights pre-tiled on host: [kp, ko, m]
    wq8 = nc.dram_tensor("wq8", [P, KO, 384], F8, kind="ExternalInput")
    wq16 = nc.dram_tensor("wq16", [P, KO, 128], F16, kind="ExternalInput")
    wg8 = nc.dram_tensor("wg8", [P, KO, 384], F8, kind="ExternalInput")
    wg16 = nc.dram_tensor("wg16", [P, KO, 128], F16, kind="ExternalInput")
    wk = nc.dram_tensor("wk", [P, KO, 512], F8, kind="ExternalInput")
    wv = nc.dram_tensor("wv", [P, KO, 512], F8, kind="ExternalInput")
    wo8 = nc.dram_tensor("wo8", [P, 2, HID], F8, kind="ExternalInput")
    wo16 = nc.dram_tensor("wo16", [P, 2, HID], F16, kind="ExternalInput")
    ddp = nc.dram_tensor("ddp", [4, P, 384], F32, kind="ExternalInput")
    qdt = nc.dram_tensor("qdt", [4, P, 512], F16, kind="ExternalInput")
    kdm = nc.dram_tensor("kdm", [4, P, 256], F16, kind="ExternalInput")
    bdec = nc.dram_tensor("bdec", [P, 4], F32, kind="ExternalInput")
    idn = nc.dram_tensor("idn", [P, P], F16, kind="ExternalInput")
    out = nc.dram_tensor("out", [S, HID], F16, kind="ExternalOutput")

    with tile.TileContext(nc) as tc:
        with tc.tile_pool(name="const", bufs=1) as cpool, \
             tc.tile_pool(name="xpool", bufs=3) as xpool, \
             tc.tile_pool(name="qkpool", bufs=2) as qkpool, \
             tc.tile_pool(name="apool", bufs=2) as apool, \
             tc.tile_pool(name="gopool", bufs=2) as gopool, \
             tc.tile_pool(name="opool", bufs=6) as opool, \
             tc.tile_pool(name="pproj", bufs=2, space="PSUM") as pproj, \
             tc.tile_pool(name="ptr", bufs=2, space="PSUM") as ptr, \
             tc.tile_pool(name="psc", bufs=2, space="PSUM") as psc, \
             tc.tile_pool(name="pout", bufs=2, space="PSUM") as pout:

            # ---- constants: fp8 weights + x8 first so fp8 matmuls start
            wq8_sb = cpool.tile([P, KO, 384], F8)
            nc.sync.dma_start(wq8_sb[:], wq8.ap())

            def load_x(blk):
                x8 = xpool.tile([P, KO, BLK], F8, tag="x8")
                nc.sync.dma_start(x8[:], xT8.ap()[blk])
                x16 = xpool.tile([P, KO, BLK], F16, tag="x16")
                nc.sync.dma_start(x16[:], xT16.ap()[blk])
                return x8, x16

            x8_0 = xpool.tile([P, KO, BLK], F8, tag="x8")
            nc.sync.dma_start(x8_0[:, :4, :], xT8.ap()[0, :, :4, :])
            nc.sync.dma_start(x8_0[:, 4:, :], xT8.ap()[0, :, 4:, :])
            wk_sb = cpool.tile([P, KO, 512], F8)
            nc.sync.dma_start(wk_sb[:], wk.ap())
            wv_sb = cpool.tile([P, KO, 512], F8)
            nc.sync.dma_start(wv_sb[:], wv.ap())
            wg8_sb = cpool.tile([P, KO, 384], F8)
            nc.sync.dma_start(wg8_sb[:], wg8.ap())
            wq16_sb = cpool.tile([P, KO, 128], F16)
            nc.sync.dma_start(wq16_sb[:], wq16.ap())
            x16_0 = xpool.tile([P, KO, BLK], F16, tag="x16")
            nc.sync.dma_start(x16_0[:], xT16.ap()[0])
            cur_x = (x8_0, x16_0)
            wg16_sb = cpool.tile([P, KO, 128], F16)
            nc.sync.dma_start(wg16_sb[:], wg16.ap())
            wo8_sb = cpool.tile([P, 2, HID], F8)
            nc.sync.dma_start(wo8_sb[:], wo8.ap())
            wo16_sb = cpool.tile([P, 2, HID], F16)
            nc.sync.dma_start(wo16_sb[:], wo16.ap())

            bd_sb = cpool.tile([P, 4], F32)
            nc.sync.dma_start(bd_sb[:], bdec.ap())
            ddp_sb = cpool.tile([P, 4, 384], F32)
            nc.sync.dma_start(ddp_sb[:], ddp.ap().rearrange("h kp i -> kp h i"))
            qdt_sb = cpool.tile([P, 4, 512], F16)
            nc.sync.dma_start(qdt_sb[:], qdt.ap().rearrange("h kp i -> kp h i"))
            kdm_sb = cpool.tile([P, 4, 256], F16)
            nc.sync.dma_start(kdm_sb[:], kdm.ap().rearrange("h kp i -> kp h i"))
            ident = cpool.tile([P, P], F16)
            nc.sync.dma_start(ident[:], idn.ap())

            kv_sb = cpool.tile([P, 4, P], F16)
            nc.vector.memset(kv_sb[:], 0.0)

            def qg_proj8(w8_sb, x8, dst, fn):
                """q/gate slots 0-2: fp8 DR."""
                for m in range(3):
                    ps_ = pproj.tile([P, 512], F32, tag="proj")
                    for kk in range(KK):
                        nc.tensor.matmul(
                            ps_[:],
                            w8_sb[:, 2 * kk:2 * kk + 2, m * P:(m + 1) * P],
                            x8[:, 2 * kk:2 * kk + 2, :],
                            start=(kk == 0), stop=(kk == KK - 1),
                            perf_mode=DR)
                    nc.scalar.activation(dst[:, m, :], ps_[:], fn, scale=DESC)

            def qg_proj16(w16_sb, x16, dst, fn):
                """q/gate slot 3: f16."""
                ps_ = pproj.tile([P, 512], F32, tag="proj")
                for ko in range(KO):
                    nc.tensor.matmul(
                        ps_[:], w16_sb[:, ko, :], x16[:, ko, :],
                        start=(ko == 0), stop=(ko == KO - 1))
                nc.scalar.activation(dst[:, 3, :], ps_[:], fn)

            for blk in range(NBLK):
                t0 = blk * BLK
                x8, x16 = cur_x
                if blk + 1 < NBLK:
                    cur_x = load_x(blk + 1)

                # ---- projections: qT, kT, gateT ([dcol, tok]); v ([tok, dcol])
                qsb = qkpool.tile([P, 4, BLK], F16, tag="qsb")
                ksb = qkpool.tile([P, 4, BLK], F16, tag="ksb")
                gsb = qkpool.tile([P, 4, BLK], F16, tag="gsb")
                vsb = qkpool.tile([P, 4, 512], F16, tag="vsb")
                qg_proj8(wq8_sb, x8, qsb, AF.Silu)
                for m in range(4):       # kT, all slots fp8
                    ps_ = pproj.tile([P, 512], F32, tag="proj")
                    for kk in range(KK):
                        nc.tensor.matmul(
                            ps_[:],
                            wk_sb[:, 2 * kk:2 * kk + 2, m * P:(m + 1) * P],
                            x8[:, 2 * kk:2 * kk + 2, :],
                            start=(kk == 0), stop=(kk == KK - 1),
                            perf_mode=DR)
                    nc.scalar.activation(ksb[:, m, :], ps_[:],
                                         AF.Silu, scale=DESC)
                for tq in range(4):      # v, token-quarter tq, all slots fp8
                    ps_ = pproj.tile([P, 512], F32, tag="proj")
                    for kk in range(KK):
                        nc.tensor.matmul(
                            ps_[:],
                            x8[:, 2 * kk:2 * kk + 2, tq * P:(tq + 1) * P],
                            wv_sb[:, 2 * kk:2 * kk + 2, :],
                            start=(kk == 0), stop=(kk == KK - 1),
                            perf_mode=DR)
                    nc.scalar.activation(vsb[:, tq, :], ps_[:],
                                         AF.Silu, scale=DESC)
                qg_proj16(wq16_sb, x16, qsb, AF.Silu)
                for m in range(3):       # gate slots 0-2 fp8
                    ps_ = pproj.tile([P, 512], F32, tag="proj")
                    for kk in range(KK):
                        nc.tensor.matmul(
                            ps_[:],
                            wg8_sb[:, 2 * kk:2 * kk + 2, m * P:(m + 1) * P],
                            x8[:, 2 * kk:2 * kk + 2, :],
                            start=(kk == 0), stop=(kk == KK - 1),
                            perf_mode=DR)
                    nc.scalar.activation(gsb[:, m, :], ps_[:], AF.Sigmoid,
                                         scale=DESC)
                qg_proj16(wg16_sb, x16, gsb, AF.Sigmoid)

                # ---- attention: 2 chunks of 256
                # q*q_decay for the whole block in one op
                qdq_all = gopool.tile([P, 4, BLK], F16, tag="qdqa")
                nc.vector.tensor_tensor(qdq_all[:], qsb[:], qdt_sb[:], MUL)
                go8 = gopool.tile([P, 2, BLK], F8, tag="go8")
                go16 = gopool.tile([P, 2, BLK], F16, tag="go16")
                for ch in range(2):
                    co = ch * C
                    first_chunk = (blk == 0 and ch == 0)
                    kns, sms = {}, {}

                    def phase_a(lh):
                        # kT chunk (both jt) into one psum; decay in one mul
                        knp = ptr.tile([P, 2, P], F16, tag="tr")
                        for jt in range(2):
                            nc.tensor.transpose(
                                knp[:, jt, :],
                                ksb[:, lh, co + jt * P:co + (jt + 1) * P],
                                ident[:])
                        kn_sb = apool.tile([P, 2, P], F16, tag="kn")
                        nc.vector.tensor_tensor(kn_sb[:], knp[:],
                                                kdm_sb[:, lh, :], MUL)
                        kns[lh] = kn_sb
                        # scoresT packed [jt0 i:0-255 | jt1 i:128-255]
                        st = psc.tile([P, 384], F32, tag="sc")
                        nc.tensor.matmul(
                            st[:, :C], ksb[:, lh, co:co + P],
                            qsb[:, lh, co:co + C], start=True, stop=True)
                        nc.tensor.matmul(
                            st[:, C:], ksb[:, lh, co + P:co + C],
                            qsb[:, lh, co + P:co + C], start=True, stop=True)
                        sm = apool.tile([P, 384], F16, tag="sm")
                        nc.vector.tensor_tensor(sm[:], st[:],
                                                ddp_sb[:, lh, :], MUL)
                        sms[lh] = sm

                    def phase_b(lh):
                        # oT[e, i] = v^T scoresT + kv^T (q * q_decay)
                        sm = sms[lh]
                        ot = psc.tile([P, C], F32, tag="sc")
                        nc.tensor.matmul(ot[:],
                                         vsb[:, 2 * ch, lh * P:(lh + 1) * P],
                                         sm[:, :C], start=True, stop=False)
                        nc.tensor.matmul(ot[:, P:],
                                         vsb[:, 2 * ch + 1, lh * P:(lh + 1) * P],
                                         sm[:, C:], start=False,
                                         stop=first_chunk)
                        if not first_chunk:
                            nc.tensor.matmul(ot[:], kv_sb[:, lh, :],
                                             qdq_all[:, lh, co:co + C],
                                             start=False, stop=True)
                        if lh < 2:   # fp8 out-pair slot: quantize in the drain
                            nc.vector.tensor_tensor(go8[:, lh, co:co + C],
                                                    ot[:],
                                                    gsb[:, lh, co:co + C], MUL)
                        else:
                            nc.vector.tensor_tensor(go16[:, lh - 2, co:co + C],
                                                    ot[:],
                                                    gsb[:, lh, co:co + C], MUL)
                        # kv <- bdecay * kv + (k kdecay)^T v
                        up = ptr.tile([P, P], F32, tag="tr")
                        for jt in range(2):
                            nc.tensor.matmul(up[:], kns[lh][:, jt, :],
                                             vsb[:, 2 * ch + jt,
                                                 lh * P:(lh + 1) * P],
                                             start=(jt == 0), stop=(jt == 1))
                        nc.vector.scalar_tensor_tensor(
                            kv_sb[:, lh, :], kv_sb[:, lh, :],
                            bd_sb[:, lh:lh + 1], up[:], MUL, ADD)

                    for lh in range(4):
                        phase_a(lh)
                        if lh > 0:
                            phase_b(lh - 1)
                    phase_b(3)

                # ---- out projection (partial over this core's 4 heads)
                # f16 slots 2-3 first (512-mov), then fp8 DR pair in halves
                for mt in range(4):
                    for nt in range(4):
                        n0 = nt * 512
                        op = pout.tile([P, 512], F32, tag="out")
                        for lh in range(2):
                            nc.tensor.matmul(
                                op[:], go16[:, lh, mt * P:(mt + 1) * P],
                                wo16_sb[:, lh, n0:n0 + 512],
                                start=(lh == 0), stop=False,
                                skip_group_check=True)
                        nc.tensor.matmul(
                            op[:], go8[:, :, mt * P:(mt + 1) * P],
                            wo8_sb[:, :, n0:n0 + 512],
                            start=False, stop=True,
                            perf_mode=DR, skip_group_check=True)
                        ob = opool.tile([P, 512], F16, tag="ob")
                        if (mt + nt) % 2 == 0:
                            nc.vector.tensor_scalar_mul(ob[:], op[:], DESC_O)
                        else:
                            nc.scalar.activation(ob[:], op[:], AF.Copy,
                                                 scale=DESC_O)
                        nc.sync.dma_start(
                            out.ap()[t0 + mt * P:t0 + (mt + 1) * P,
                                     n0:n0 + 512],
                            ob[:])

    nc.compile()
    return nc


def _get_program():
    global _PROG
    if _PROG is None:
        _PROG = _build_program()
    return _PROG


def _q8(a, scale):
    import ml_dtypes
    return (np.asarray(a, np.float32) * scale).astype(ml_dtypes.float8_e4m3)


def _heads_for_quad(qd):
    return [2 * qd, 2 * qd + 1, 8 + qd, 12 + qd]


def _prep_shared(x, w_qkv, w_gate, w_out, slopes):
    shared = {}
    for b in range(B):
        xT = np.ascontiguousarray(x[b].T)  # [HID, S]
        shared[f"x8_{b}"] = np.ascontiguousarray(
            _q8(xT, SX).reshape(KO, P, NBLK, BLK).transpose(2, 1, 0, 3))
        shared[f"x16_{b}"] = np.ascontiguousarray(
            xT.astype(np.float16).reshape(KO, P, NBLK, BLK).transpose(2, 1, 0, 3))
    shared["wqkv"] = np.asarray(w_qkv, np.float32)
    shared["wg"] = np.asarray(w_gate, np.float32)
    shared["wo"] = np.asarray(w_out, np.float32)
    return shared


def _prep_core_inputs(shared, slopes, core):
    b, qd = core // 4, core % 4
    heads = _heads_for_quad(qd)
    s = np.asarray(slopes, dtype=np.float32).reshape(H)[heads]  # [4]

    def cols(w, hs):
        return np.concatenate([w[:, h * D:(h + 1) * D] for h in hs], axis=1)

    def tile8(w2):  # [HID, n] fp8-scaled -> [kp, ko, n]
        n = w2.shape[1]
        return np.ascontiguousarray(
            _q8(w2, SW).reshape(KO, P, n).transpose(1, 0, 2))

    def tile16(w2):
        n = w2.shape[1]
        return np.ascontiguousarray(
            w2.astype(np.float16).reshape(KO, P, n).transpose(1, 0, 2))

    wq = shared["wqkv"][:, :2048]
    wk = shared["wqkv"][:, 2048:4096]
    wv = shared["wqkv"][:, 4096:]
    wq8_c = tile8(cols(wq, heads[:3]))
    wq16_c = tile16(cols(wq, heads[3:]))
    wg8_c = tile8(cols(shared["wg"], heads[:3]))
    wg16_c = tile16(cols(shared["wg"], heads[3:]))
    wk_c = tile8(cols(wk, heads))
    wv_c = tile8(cols(wv, heads))
    # w_out rows for this core's heads (x SW) -> [kp, kh, n]
    wo_rows = np.stack([shared["wo"][h * D:(h + 1) * D] * SW for h in heads])
    wo8_c = np.ascontiguousarray(
        _q8(wo_rows[:2], 1.0).transpose(1, 0, 2))       # [P, 2, HID] fp8
    wo16_c = np.ascontiguousarray(
        wo_rows[2:].astype(np.float16).transpose(1, 0, 2))  # [P, 2, HID]

    pos = np.arange(C, dtype=np.float32)
    idx = pos[:, None] - pos[None, :]                      # [i, j] -> i - j
    ddp = np.empty((4, P, 384), dtype=np.float32)
    qdt = np.empty((4, P, 512), dtype=np.float16)
    kdm = np.empty((4, P, 256), dtype=np.float16)
    bdec = np.empty((P, 4), dtype=np.float32)
    for lh in range(4):
        sh = np.float64(s[lh])
        m = np.where(idx >= 0, np.exp(-sh * idx) * SGO, 0.0)   # [i, j]
        mt_ = m.T.reshape(2, P, C).astype(np.float32)          # [jt, j, i]
        ddp[lh, :, :C] = mt_[0]
        ddp[lh, :, C:] = mt_[1][:, P:]
        qd1 = (np.exp(-sh * (pos + 1.0)) * SGO).astype(np.float16)
        qdt[lh] = np.broadcast_to(np.concatenate([qd1, qd1])[None, :], (P, 512))
        kcol = np.exp(-sh * (C - 1.0 - pos)).astype(np.float16).reshape(2, P)
        kdm[lh, :, :P] = np.broadcast_to(kcol[0][:, None], (P, P))
        kdm[lh, :, P:] = np.broadcast_to(kcol[1][:, None], (P, P))
        bdec[:, lh] = np.float32(np.exp(-sh * C))

    return {
        "xT8": shared[f"x8_{b}"], "xT16": shared[f"x16_{b}"],
        "wq8": wq8_c, "wq16": wq16_c, "wg8": wg8_c, "wg16": wg16_c,
        "wk": wk_c, "wv": wv_c, "wo8": wo8_c, "wo16": wo16_c,
        "ddp": ddp, "qdt": qdt, "kdm": kdm,
        "bdec": np.ascontiguousarray(bdec),
        "idn": np.eye(P, dtype=np.float16),
    }


def kernel(x, w_qkv, w_gate, w_out, slopes, _trace=False, _result_holder=None):
    from concourse.bass_utils import run_bass_kernel_spmd

    x = np.asarray(x, dtype=np.float32)
    w_qkv = np.asarray(w_qkv, dtype=np.float32)
    w_gate = np.asarray(w_gate, dtype=np.float32)
    w_out = np.asarray(w_out, dtype=np.float32)

    nc = _get_program()
    shared = _prep_shared(x, w_qkv, w_gate, w_out, slopes)
    in_maps = [_prep_core_inputs(shared, slopes, c) for c in range(NCORES)]
    res = run_bass_kernel_spmd(nc, in_maps, core_ids=list(range(NCORES)),
                               trace=_trace)
    if _result_holder is not None:
        _result_holder.append(res)

    out = np.zeros((B, S, HID), dtype=np.float32)
    for c in range(NCORES):
        out[c // 4] += res.results[c]["out"].astype(np.float32)
    return out
